# revision 22
# baseline (speedup 1.0000x reference)
"""Trainium2 Bass kernel for batched CRF negative-log-likelihood (nn_CRF).

Strategy (data-parallel over batch across 8 cores, B_loc=256/core):
  - Exact 4-state reduction of the 6-state CRF (START/STOP rows are -10000 =>
    exp underflows to exactly 0 in f32; first/last steps handled specially).
  - Forward pass in the exp domain: per-step 4x4 positive matrices
    V_t[n,p] = exp(f_t[n] + Tr[n,p] + g_t[p]*M[n,p] - kappa); the T-scan is
    computed as 32 chunk-parallel 4x4 matrix-product chains (TT-mul + strided
    reduce on the vector engine), periodically renormalized (log accumulated).
  - Gold path score = sum_t argpre[cell_t] computed with a one-hot mask and a
    mul+reduce on the same pre-exponential tile (the -kappa*T offsets cancel
    exactly between forward and gold).
"""

import os
import sys
import numpy as np
from contextlib import ExitStack

for _p in ("/opt/trn_rl_repo",):
    if _p not in sys.path:
        sys.path.insert(0, _p)

import concourse.bass as bass
import concourse.tile as tile
from concourse import bacc, mybir
from concourse.bass_utils import run_bass_kernel_spmd

F32 = mybir.dt.float32
BF16 = mybir.dt.bfloat16
I32 = mybir.dt.int32
AF = mybir.ActivationFunctionType
OP = mybir.AluOpType

K = 4
NT = 6
START, STOP = 4, 5

# ---------------- configuration ----------------
class Cfg:
    def __init__(self, B_loc=256, T=2048, NCH=32, TB=4, RB_EVERY=2, SRENORM=8):
        self.B_loc = B_loc          # batches per core
        self.T = T
        self.NH = B_loc // 128      # batch "halves" stacked along free dim
        self.NCH = NCH              # chunks per batch (chunk-parallel scan)
        self.L = T // NCH           # steps per chunk
        self.TB = TB                # time-block (steps per streamed block)
        self.NBLK = self.L // TB
        self.RB_EVERY = RB_EVERY    # renormalize Cmat every RB_EVERY blocks
        self.SRENORM = SRENORM      # renormalize s every SRENORM chunks
        assert B_loc % 128 == 0 and T % NCH == 0 and self.L % TB == 0


# ------------- host-side constant prep -------------
def host_consts(transitions, w_shift_in, bias_no, bias_with, w_with_out,
                w_no_out, multiplier):
    Tr = np.asarray(transitions, np.float32)
    mult = np.asarray(multiplier, np.float64)
    # softmax over dim 0 (columns), diagonal then set to -1
    e = np.exp(mult - mult.max(axis=0, keepdims=True))
    Mm = (e / e.sum(axis=0, keepdims=True)).astype(np.float32)
    np.fill_diagonal(Mm, -1.0)

    Tr44 = Tr[:K, :K]
    kappa = float(np.log(np.exp(Tr44.astype(np.float64)).sum(axis=1).mean()))
    consts = np.zeros((128, 96), np.float32)
    consts[:, 0:16] = Mm.reshape(-1)                      # M[n,p] row-major
    consts[:, 16:32] = (Tr44 - kappa).reshape(-1)         # Trkap[n,p]
    consts[:, 32:36] = Tr[:K, START] - kappa              # startColKappa[n]
    consts[:, 36:52] = np.arange(16, dtype=np.float32)    # iota16
    consts[:, 52:68] = np.eye(4, dtype=np.float32).reshape(-1)  # identity
    consts[:, 68:72] = np.exp(Tr[STOP, :K])               # estop
    consts[:, 72:76] = Tr[STOP, :K]                       # stop_row
    consts[:, 76] = float(np.asarray(bias_with).reshape(-1)[0])
    consts[:, 77] = float(np.asarray(bias_no).reshape(-1)[0])
    return consts, dict(
        kappa=kappa,
        wsh=np.asarray(w_shift_in, np.float32),
        b_no=float(np.asarray(bias_no).reshape(-1)[0]),
        b_with=float(np.asarray(bias_with).reshape(-1)[0]),
        w_w=np.asarray(w_with_out, np.float32),
        w_n=np.asarray(w_no_out, np.float32),
    )


# ------------- device program -------------
def build_program(cfg: Cfg, scal, debug=False):
    """Build the Bass program. `scal` carries the python-scalar constants that
    are baked in as immediates (wsh/b_no/b_with/w_w/w_n)."""
    nc = bacc.Bacc("TRN2", target_bir_lowering=False, debug=debug)
    B, T, NH, NCH, L, TB, NBLK = (cfg.B_loc, cfg.T, cfg.NH, cfg.NCH, cfg.L,
                                  cfg.TB, cfg.NBLK)
    NSL = NH * NCH  # slots per partition

    # inputs are host-packed per block: [NBLK, B, NCH, TB, ...]
    feats_d = nc.dram_tensor("feats", [NBLK, B, NCH, TB, K], F32, kind="ExternalInput")
    bias_d = nc.dram_tensor("bias", [NBLK, B, NCH, TB], F32, kind="ExternalInput")
    t1_d = nc.dram_tensor("t1", [NBLK, B, NCH, TB], I32, kind="ExternalInput")
    t0_d = nc.dram_tensor("t0", [NBLK, B, NCH, TB], I32, kind="ExternalInput")
    consts_d = nc.dram_tensor("consts", [128, 96], F32, kind="ExternalInput")
    out_d = nc.dram_tensor("nll", [B], F32, kind="ExternalOutput")

    def blk_view(d, j, trail):
        return d.ap()[j].rearrange("(h p) c i" + (" n" if trail else "") +
                                   " -> p h c i" + (" n" if trail else ""), p=128)
    ov = out_d.ap().rearrange("(h p) -> p h", p=128)

    wsh, w_w, w_n = scal["wsh"], scal["w_w"], scal["w_n"]
    b_no, b_with = scal["b_no"], scal["b_with"]

    with tile.TileContext(nc) as tc, ExitStack() as ctx:
        persist = ctx.enter_context(tc.tile_pool(name="persist", bufs=1))
        stream = ctx.enter_context(tc.tile_pool(name="stream", bufs=2))
        work = ctx.enter_context(tc.tile_pool(name="work", bufs=2))
        big = ctx.enter_context(tc.tile_pool(name="big", bufs=2))
        single = ctx.enter_context(tc.tile_pool(name="single", bufs=1))

        consts = persist.tile([128, 96], F32)
        nc.sync.dma_start(consts[:], consts_d.ap())
        def cst(lo, hi, shape_prefix_dims, dims):
            """consts[:, lo:hi] broadcast to [128, *shape_prefix_dims, *dims]."""
            a = consts[:, lo:hi]
            if len(dims) == 2:
                a = a.rearrange("p (n q) -> p n q", q=dims[1])
            for _ in shape_prefix_dims:
                a = a.unsqueeze(1)
            return a.broadcast_to([128] + list(shape_prefix_dims) + list(dims))

        Cmat = persist.tile([128, NSL, 16], F32)      # chunk matrices, col-major (k,p') -> 4*p'+k
        logacc = persist.tile([128, NSL], F32)
        goldacc = persist.tile([128, NH, NBLK], F32)
        slogsum = persist.tile([128, NH], F32)

        # init: Cmat = I per slot, logacc = 0
        nc.vector.tensor_copy(Cmat[:], cst(52, 68, [NSL], [16]))
        nc.vector.memset(logacc[:], 0.0)
        nc.vector.memset(slogsum[:], 0.0)

        HCI = NH * NCH * TB  # flattened (h, c, i) block index
        for j in range(NBLK):
            # ---- DMA loads (tiles kept flat; all compute APs <= 3 free dims) ----
            feats_t = stream.tile([128, HCI, K], F32, tag="feats")
            nc.sync.dma_start(feats_t[:], blk_view(feats_d, j, True))
            bias_t = stream.tile([128, HCI], F32, tag="bias")
            nc.sync.dma_start(bias_t[:], blk_view(bias_d, j, False))
            t1_t = stream.tile([128, HCI], I32, tag="t1")
            nc.sync.dma_start(t1_t[:], blk_view(t1_d, j, False))
            t0_t = stream.tile([128, HCI], I32, tag="t0")
            nc.sync.dma_start(t0_t[:], blk_view(t0_d, j, False))

            # ---- gates ----
            tanhW = work.tile([128, HCI, K], F32, tag="tanhW")
            tanhN = work.tile([128, HCI, K], F32, tag="tanhN")
            for p in range(K):
                nc.scalar.activation(tanhW[:, :, p], bias_t[:],
                                     AF.Tanh, bias=consts[:, 76:77], scale=float(wsh[p]))
                nc.scalar.activation(tanhN[:, :, p], bias_t[:],
                                     AF.Tanh, bias=consts[:, 77:78], scale=float(wsh[p]))
            gw = work.tile([128, HCI, K], F32, tag="gw")
            gn = work.tile([128, HCI, K], F32, tag="gn")
            for p in range(K):
                nc.scalar.mul(gw[:, :, p], tanhW[:, :, p], float(w_w[p]))
                nc.scalar.mul(gn[:, :, p], tanhN[:, :, p], float(w_n[p]))
            mask = work.tile([128, HCI], F32, tag="mask")
            nc.vector.tensor_scalar(mask[:], bias_t[:], 0.5, None, OP.is_gt)
            # g computed in place: gw <- (gw-gn)*mask ; gn <- gn + gw  (= g)
            nc.vector.tensor_sub(gw[:], gw[:], gn[:])
            nc.vector.tensor_tensor(gw[:], gw[:],
                                    mask[:].unsqueeze(2).broadcast_to((128, HCI, K)),
                                    OP.mult)
            nc.vector.tensor_add(gn[:], gn[:], gw[:])
            g_t = gn

            # ---- argpre[n,p] = g[p]*M[n,p] + Trkap[n,p] + f[n] ----
            argpre = single.tile([128, HCI, K, K], F32, tag="argpre")
            nc.vector.tensor_tensor(
                argpre[:],
                g_t[:].unsqueeze(2).broadcast_to((128, HCI, K, K)),
                cst(0, 16, [HCI], [K, K]), OP.mult)
            nc.vector.tensor_add(argpre[:], argpre[:], cst(16, 32, [HCI], [K, K]))
            nc.vector.tensor_tensor(
                argpre[:], argpre[:],
                feats_t[:].unsqueeze(3).broadcast_to((128, HCI, K, K)),
                OP.add)
            if j == 0:
                # special first step: argpre[c=0,i=0,n,p] = f[0,n] + Tr[n,START]-kappa
                ap0 = argpre[:].rearrange("p (h x) n q -> p h x n q", h=NH)[:, :, 0]
                f0 = feats_t[:].rearrange("p (h x) n -> p h x n", h=NH)[:, :, 0, :]
                nc.vector.tensor_tensor(
                    ap0, f0.unsqueeze(3).broadcast_to((128, NH, K, K)),
                    consts[:, 32:36].unsqueeze(1).unsqueeze(3)
                        .broadcast_to((128, NH, K, K)),
                    OP.add)

            # ---- V = exp(argpre) ----
            Vt = big.tile([128, HCI, K, K], F32, tag="V")
            nc.scalar.activation(Vt[:].rearrange("p x n q -> p (x n q)"),
                                 argpre[:].rearrange("p x n q -> p (x n q)"),
                                 AF.Exp)

            # ---- gold: cell = 4*t1 + t0 ; goldacc[j] = sum(argpre * onehot) ----
            cell_i = work.tile([128, HCI], I32, tag="cell_i")
            nc.vector.scalar_tensor_tensor(cell_i[:], t1_t[:], 4, t0_t[:],
                                           OP.mult, OP.add)
            cellf = work.tile([128, HCI], F32, tag="cellf")
            nc.vector.tensor_copy(cellf[:], cell_i[:])
            prod = single.tile([128, HCI, 16], F32, tag="prod")
            nc.vector.tensor_tensor(
                prod[:], cellf[:].unsqueeze(2).broadcast_to((128, HCI, 16)),
                cst(36, 52, [HCI], [16]), OP.is_equal)
            nc.vector.tensor_tensor(
                prod[:], prod[:],
                argpre[:].rearrange("p x n q -> p x (n q)"), OP.mult)
            nc.vector.reduce_sum(
                goldacc[:, :, j],
                prod[:].rearrange("p (h x) q -> p h (x q)", h=NH),
                axis=mybir.AxisListType.X)

            # ---- chain: Cmat <- V_i @ Cmat for each step i ----
            Vs = Vt[:].rearrange("p (s i) n k -> p s i n k", i=TB)
            for i in range(TB):
                tmp = single.tile([128, NSL, K, K, K], F32, tag="ctmp")
                Ck = Cmat[:].rearrange("p s (q k) -> p s q k", k=K)
                for n in range(K):
                    nc.vector.tensor_tensor(
                        tmp[:, :, n],
                        Vs[:, :, i, n, :].unsqueeze(2).broadcast_to((128, NSL, K, K)),
                        Ck, OP.mult)
                nc.vector.reduce_sum(
                    Cmat[:].rearrange("p s (q n) -> p s n q", n=K),
                    tmp[:].rearrange("p s n q k -> p (s n q) k"),
                    axis=mybir.AxisListType.X)

            # ---- renorm Cmat ----
            if (j + 1) % cfg.RB_EVERY == 0 or j == NBLK - 1:
                m_t = work.tile([128, NSL], F32, tag="m")
                nc.vector.reduce_max(m_t[:], Cmat[:], axis=mybir.AxisListType.X)
                r_t = work.tile([128, NSL], F32, tag="r")
                nc.vector.reciprocal(r_t[:], m_t[:])
                nc.vector.tensor_tensor(
                    Cmat[:], Cmat[:],
                    r_t[:].unsqueeze(2).broadcast_to((128, NSL, 16)), OP.mult)
                lnm = work.tile([128, NSL], F32, tag="lnm")
                nc.scalar.activation(lnm[:], m_t[:], AF.Ln)
                nc.vector.tensor_add(logacc[:], logacc[:], lnm[:])

        # ---------------- final combine ----------------
        s_t = persist.tile([128, NH, K], F32)
        # s = column 0 of chunk-0 matrix  (C stored col-major: col p'=0 = first 4)
        nc.vector.tensor_copy(
            s_t[:], Cmat[:].rearrange("p (h c) q -> p h c q", h=NH)[:, :, 0, 0:K])
        for c in range(1, NCH):
            stmp = work.tile([128, NH, K, K], F32, tag="stmp")
            Cc = Cmat[:].rearrange("p (h c) (q n) -> p h c n q", h=NH, n=K)[:, :, c]
            nc.vector.tensor_tensor(
                stmp[:], Cc,
                s_t[:].unsqueeze(2).broadcast_to((128, NH, K, K)), OP.mult)
            nc.vector.reduce_sum(s_t[:], stmp[:], axis=mybir.AxisListType.X)
            if c % cfg.SRENORM == 0:
                m2 = work.tile([128, NH], F32, tag="m2")
                nc.vector.reduce_max(m2[:], s_t[:], axis=mybir.AxisListType.X)
                r2 = work.tile([128, NH], F32, tag="r2")
                nc.vector.reciprocal(r2[:], m2[:])
                nc.vector.tensor_tensor(
                    s_t[:], s_t[:], r2[:].unsqueeze(2).broadcast_to((128, NH, K)),
                    OP.mult)
                ln2 = work.tile([128, NH], F32, tag="ln2")
                nc.scalar.activation(ln2[:], m2[:], AF.Ln)
                nc.vector.tensor_add(slogsum[:], slogsum[:], ln2[:])

        # fwd = ln(sum_n s[n]*estop[n]) + sum(logacc) + slogsum
        sdot = work.tile([128, NH, K], F32, tag="sdot")
        nc.vector.tensor_tensor(sdot[:], s_t[:], cst(68, 72, [NH], [K]), OP.mult)
        dotv = work.tile([128, NH], F32, tag="dotv")
        nc.vector.reduce_sum(dotv[:], sdot[:], axis=mybir.AxisListType.X)
        fwdp = work.tile([128, NH], F32, tag="fwdp")
        nc.scalar.activation(fwdp[:], dotv[:], AF.Ln)
        lsum = work.tile([128, NH], F32, tag="lsum")
        nc.vector.reduce_sum(lsum[:], logacc[:].rearrange("p (h c) -> p h c", h=NH),
                             axis=mybir.AxisListType.X)

        # gold total + stop fix
        gtot = work.tile([128, NH], F32, tag="gtot")
        nc.vector.reduce_sum(gtot[:], goldacc[:], axis=mybir.AxisListType.X)
        tl = work.tile([128, NH], I32, tag="tl")
        nc.sync.dma_start(
            tl[:], t1_d.ap()[NBLK - 1, :, NCH - 1, TB - 1].rearrange(
                "(h p) -> p h", p=128))
        tlf = work.tile([128, NH], F32, tag="tlf")
        nc.vector.tensor_copy(tlf[:], tl[:])
        ohl = work.tile([128, NH, K], F32, tag="ohl")
        nc.vector.tensor_tensor(ohl[:],
                                tlf[:].unsqueeze(2).broadcast_to((128, NH, K)),
                                cst(36, 40, [NH], [K]), OP.is_equal)
        sfix = work.tile([128, NH, K], F32, tag="sfix")
        nc.vector.tensor_tensor(sfix[:], ohl[:], cst(72, 76, [NH], [K]), OP.mult)
        fixv = work.tile([128, NH], F32, tag="fixv")
        nc.vector.reduce_sum(fixv[:], sfix[:], axis=mybir.AxisListType.X)

        nll = work.tile([128, NH], F32, tag="nll")
        nc.vector.tensor_add(nll[:], fwdp[:], lsum[:])
        nc.vector.tensor_add(nll[:], nll[:], slogsum[:])
        nc.vector.tensor_sub(nll[:], nll[:], gtot[:])
        nc.vector.tensor_sub(nll[:], nll[:], fixv[:])
        nc.sync.dma_start(ov, nll[:])

    nc.compile()
    return nc


def host_pack(feats, bias, tags, cfg: Cfg):
    """Repack [B,T,...] into block-major [NBLK, B, NCH, TB, ...] layouts."""
    B, T = bias.shape
    NCH, NBLK, TB = cfg.NCH, cfg.NBLK, cfg.TB

    def pack(x):
        trail = x.shape[2:]
        xr = x.reshape(B, NCH, NBLK, TB, *trail)
        order = (2, 0, 1, 3) + tuple(range(4, 4 + len(trail)))
        return np.ascontiguousarray(xr.transpose(*order))

    t0 = np.empty_like(tags)
    t0[:, 1:] = tags[:, :-1]
    t0[:, 0] = 0
    return (pack(np.ascontiguousarray(feats[:, :, :K])), pack(bias),
            pack(tags), pack(t0))


_CACHE = {}


def _get_program(cfg_key, cfg, scal):
    if cfg_key not in _CACHE:
        _CACHE[cfg_key] = build_program(cfg, scal)
    return _CACHE[cfg_key]


def kernel(feats, bias, tags, transitions, w_shift_in, bias_no, bias_with,
           w_with_out, w_no_out, multiplier):
    feats = np.ascontiguousarray(np.asarray(feats, np.float32))
    bias = np.ascontiguousarray(np.asarray(bias, np.float32))
    tags = np.ascontiguousarray(np.asarray(tags).astype(np.int32))
    B, T, _ = feats.shape
    n_cores = 8
    B_loc = B // n_cores
    cfg = Cfg(B_loc=B_loc, T=T)
    consts, scal = host_consts(transitions, w_shift_in, bias_no, bias_with,
                               w_with_out, w_no_out, multiplier)
    key = (B_loc, T, tuple(consts[0, :76].tobytes()),)
    nc = _get_program((B_loc, T, consts[0, :96].tobytes()), cfg, scal)

    in_maps = []
    for k in range(n_cores):
        sl = slice(k * B_loc, (k + 1) * B_loc)
        fr, br, t1r, t0r = host_pack(feats[sl], bias[sl], tags[sl], cfg)
        in_maps.append(dict(feats=fr, bias=br, t1=t1r, t0=t0r, consts=consts))
    trace = bool(int(os.environ.get("BASS_KERNEL_TRACE", "0")))
    res = run_bass_kernel_spmd(nc, in_maps, core_ids=list(range(n_cores)),
                               trace=trace)
    global LAST_EXEC_NS
    LAST_EXEC_NS = res.exec_time_ns
    out = np.concatenate([r["nll"] for r in res.results], axis=0)
    return out.astype(np.float32)


LAST_EXEC_NS = None


def bench(inputs, iters=10):
    """Time device execution with device-resident inputs (excludes H2D)."""
    import time
    import jax
    from jax.sharding import Mesh, PartitionSpec, NamedSharding
    from jax.experimental.shard_map import shard_map
    from concourse import bass2jax, mybir as _mb

    feats = np.ascontiguousarray(np.asarray(inputs["feats"], np.float32))
    bias = np.ascontiguousarray(np.asarray(inputs["bias"], np.float32))
    tags = np.ascontiguousarray(np.asarray(inputs["tags"]).astype(np.int32))
    B, T, _ = feats.shape
    n_cores = 8
    B_loc = B // n_cores
    cfg = Cfg(B_loc=B_loc, T=T)
    consts, scal = host_consts(*[inputs[k] for k in
                                 ("transitions", "w_shift_in", "bias_no",
                                  "bias_with", "w_with_out", "w_no_out",
                                  "multiplier")])
    nc = _get_program((B_loc, T, consts[0, :96].tobytes()), cfg, scal)

    bass2jax.install_neuronx_cc_hook()
    partition_name = nc.partition_id_tensor.name if nc.partition_id_tensor else None
    in_names, out_names, out_avals = [], [], []
    for alloc in nc.m.functions[0].allocations:
        if not isinstance(alloc, mybir.MemoryLocationSet):
            continue
        name = alloc.memorylocations[0].name
        if alloc.kind == "ExternalInput":
            if name != partition_name:
                in_names.append(name)
        elif alloc.kind == "ExternalOutput":
            out_names.append(name)
            out_avals.append(jax.core.ShapedArray(tuple(alloc.tensor_shape),
                                                  mybir.dt.np(alloc.dtype)))
    n_params = len(in_names)
    n_outs = len(out_names)
    zero_outs = [np.zeros(av.shape, av.dtype) for av in out_avals]
    in_names_full = list(in_names) + list(out_names)
    if partition_name is not None:
        in_names_full.append(partition_name)
    dbg_extra = {}
    if nc.dbg_addr is not None:
        dbg_extra[nc.dbg_addr.name] = np.zeros((1, 2), np.uint32)

    def _body(*args):
        operands = list(args)
        if partition_name is not None:
            operands.append(bass2jax.partition_id_tensor())
        return tuple(bass2jax._bass_exec_p.bind(
            *operands, out_avals=tuple(out_avals), in_names=tuple(in_names_full),
            out_names=tuple(out_names), lowering_input_output_aliases=(),
            sim_require_finite=True, sim_require_nnan=True, nc=nc))

    devices = jax.devices()[:n_cores]
    mesh = Mesh(np.asarray(devices), ("core",))
    spec = PartitionSpec("core")
    donate = tuple(range(n_params, n_params + n_outs))
    sharded = jax.jit(shard_map(_body, mesh=mesh,
                                in_specs=(spec,) * (n_params + n_outs),
                                out_specs=(spec,) * n_outs,
                                check_rep=False),
                      donate_argnums=donate, keep_unused=True)
    # build concatenated global inputs
    per_core = []
    for k in range(n_cores):
        sl = slice(k * B_loc, (k + 1) * B_loc)
        fr, br, t1r, t0r = host_pack(feats[sl], bias[sl], tags[sl], cfg)
        m = dict(feats=fr, bias=br, t1=t1r, t0=t0r, consts=consts, **dbg_extra)
        per_core.append([np.asarray(m[nm]) for nm in in_names])
    concat_in = [np.concatenate([per_core[c][i] for c in range(n_cores)], axis=0)
                 for i in range(n_params)]
    concat_zeros = [np.zeros((n_cores * z.shape[0], *z.shape[1:]), z.dtype)
                    for z in zero_outs]
    sh = NamedSharding(mesh, spec)
    dev_in = [jax.device_put(a, sh) for a in concat_in]

    def run_once():
        zs = [jax.device_put(z, sh) for z in concat_zeros]
        out = sharded(*dev_in, *zs)
        jax.block_until_ready(out)
        return out

    run_once()
    times = []
    for _ in range(iters):
        zs = [jax.device_put(z, sh) for z in concat_zeros]
        jax.block_until_ready(zs)
        t0 = time.perf_counter()
        out = sharded(*dev_in, *zs)
        jax.block_until_ready(out)
        times.append(time.perf_counter() - t0)
    times = np.array(times) * 1e9
    print(f"bench: min={times.min():.0f}ns mean={times.mean():.0f}ns "
          f"median={np.median(times):.0f}ns over {iters} iters")
    return float(np.min(times))


if __name__ == "__main__":
    # quick smoke test with random data
    rng = np.random.default_rng(0)
    B, T = 2048, 2048
    inputs = dict(
        feats=rng.standard_normal((B, T, NT), dtype=np.float32),
        bias=rng.random((B, T), dtype=np.float32),
        tags=rng.integers(0, K, (B, T)).astype(np.int32),
        transitions=rng.standard_normal((NT, NT)).astype(np.float32),
        w_shift_in=rng.standard_normal(K).astype(np.float32),
        bias_no=rng.standard_normal(1).astype(np.float32),
        bias_with=rng.standard_normal(1).astype(np.float32),
        w_with_out=rng.standard_normal(K).astype(np.float32),
        w_no_out=rng.standard_normal(K).astype(np.float32),
        multiplier=rng.standard_normal((K, K)).astype(np.float32),
    )
    out = kernel(**inputs)
    print(out.shape, out[:4])
